# revision 6
# baseline (speedup 1.0000x reference)
"""Trainium2 Bass kernel for contrastive loss with cosine hard-negative mining.

Reference math (B=4096, D=1024):
    loss = mean(relu(pos - i2t_neg + m)) + mean(relu(pos - t2i_neg + m))
    pos      = 1 - cos(img_b, txt_b)
    i2t_neg  = min_k (1 - cos(img_b, txt[cand_txt[b,k]]))   (best-of-2 mining;
    t2i_neg  = min_k (1 - cos(txt_b, img[cand_img[b,k]]))    the mined distance
                                                             IS the neg distance)
    i2t_cosine = t2i_cosine = cos(img_b, txt_b)  (unclamped)

Sharding: data-parallel over batch across 8 cores (512 rows each).
Candidate rows are fetched with one dma_gather per (modality, 128-row group)
from bf16 copies of the embedding tables replicated in each core's HBM
(bf16 halves the gather traffic; the rounding noise averages out of the
loss, rel err ~1e-6, and the exported cosine stays full f32 because the
anchor path is f32). Anchor norms + the img.txt dot run in f32 on
ACT/DVE fused multiply-reduce passes; candidate dots/norms run in bf16.
Outputs per core: [128, 12] = 4 groups x (cosine, i2t relu-loss, t2i
relu-loss). Host concatenates cosine shards and sums the loss partials.
"""

import sys

sys.path.insert(0, "/opt/trn_rl_repo")

import numpy as np

B, D = 4096, 1024
NCORES = 8
RPC = B // NCORES  # rows per core (512)
P = 128
G = RPC // P  # partition groups per core (4)
MARGIN = 0.2
EPS = 1e-8

# stats tile column indices (quantities accumulated per group)
W12, WT0, WT1, WI0, WI1, NIMG, NTXT, GT0, GT1, GI0, GI1 = range(11)

_CACHE = {}


def _build():
    from concourse import bacc, bass, mybir, library_config
    import concourse.tile as tile

    dt = mybir.dt
    f32 = dt.float32
    bf16 = dt.bfloat16
    Alu = mybir.AluOpType
    Act = mybir.ActivationFunctionType

    nc = bacc.Bacc("TRN2", target_bir_lowering=False, debug=False)

    img_bf = nc.dram_tensor("img_bf", [B, D], bf16, kind="ExternalInput")
    txt_bf = nc.dram_tensor("txt_bf", [B, D], bf16, kind="ExternalInput")
    img_anchor = nc.dram_tensor("img_anchor", [RPC, D], f32, kind="ExternalInput")
    txt_anchor = nc.dram_tensor("txt_anchor", [RPC, D], f32, kind="ExternalInput")
    # dma_gather wrapped index layout, one [128, 16] block per group
    gidx_img = nc.dram_tensor("gidx_img", [P, G * 16], dt.int16, kind="ExternalInput")
    gidx_txt = nc.dram_tensor("gidx_txt", [P, G * 16], dt.int16, kind="ExternalInput")
    out = nc.dram_tensor("out", [P, 3 * G], f32, kind="ExternalOutput")

    with tile.TileContext(nc) as tc:
        with (
            tc.tile_pool(name="anchors", bufs=3) as pa,
            tc.tile_pool(name="gathers", bufs=3) as pg,
            tc.tile_pool(name="scratch", bufs=4) as ps,
            tc.tile_pool(name="small", bufs=1) as psm,
        ):
            nc.gpsimd.load_library(library_config.mlp)
            ii = psm.tile([P, G * 16], dt.int16, tag="ii")
            it = psm.tile([P, G * 16], dt.int16, tag="it")
            nc.sync.dma_start(ii[:], gidx_img[:])
            nc.sync.dma_start(it[:], gidx_txt[:])

            stats = psm.tile([P, 11, G], f32, tag="stats")

            for g in range(G):
                a_img = pa.tile([P, D], f32, tag="a_img")
                a_txt = pa.tile([P, D], f32, tag="a_txt")
                nc.sync.dma_start(a_img[:], img_anchor[g * P : (g + 1) * P, :])
                nc.sync.dma_start(a_txt[:], txt_anchor[g * P : (g + 1) * P, :])

                gt = pg.tile([P, 2, D], bf16, tag="gt")
                gi = pg.tile([P, 2, D], bf16, tag="gi")
                nc.gpsimd.dma_gather(
                    gt[:], txt_bf[:], it[:, g * 16 : (g + 1) * 16], 2 * P, 2 * P, D
                )
                nc.gpsimd.dma_gather(
                    gi[:], img_bf[:], ii[:, g * 16 : (g + 1) * 16], 2 * P, 2 * P, D
                )

                # bf16 anchor copies for the candidate dots (DVE cast copy)
                ab_img = pa.tile([P, D], bf16, tag="ab_img")
                ab_txt = pa.tile([P, D], bf16, tag="ab_txt")
                nc.vector.tensor_copy(ab_img[:], a_img[:])
                nc.vector.tensor_copy(ab_txt[:], a_txt[:])

                # ACT: square + free-dim accumulate (f32 anchors, 3 bf16 cands)
                for src, q, sdt in (
                    (a_img[:], NIMG, f32),
                    (a_txt[:], NTXT, f32),
                    (gt[:, 0, :], GT0, bf16),
                    (gt[:, 1, :], GT1, bf16),
                    (gi[:, 0, :], GI0, bf16),
                ):
                    scr = ps.tile([P, D], sdt, tag=f"scr_act_{sdt.name}")
                    nc.scalar.activation(
                        scr[:], src, Act.Square,
                        accum_out=stats[:, q, g : g + 1],
                    )

                # DVE: fused multiply-reduce dots (w12 in f32, cands in bf16)
                for x, y, q, sdt in (
                    (a_img[:], a_txt[:], W12, f32),
                    (ab_img[:], gt[:, 0, :], WT0, bf16),
                    (ab_img[:], gt[:, 1, :], WT1, bf16),
                    (ab_txt[:], gi[:, 0, :], WI0, bf16),
                    (ab_txt[:], gi[:, 1, :], WI1, bf16),
                    (gi[:, 1, :], gi[:, 1, :], GI1, bf16),
                ):
                    scr = ps.tile([P, D], sdt, tag=f"scr_dve_{sdt.name}")
                    nc.vector.scalar_tensor_tensor(
                        out=scr[:],
                        in0=x,
                        scalar=1.0,
                        in1=y,
                        op0=Alu.mult,
                        op1=Alu.mult,
                        accum_out=stats[:, q, g : g + 1],
                    )

            margin = psm.tile([P, 1], f32, tag="margin")
            nc.vector.memset(margin[:], MARGIN)

            # ---- epilogue on [P, *, G] slices of stats ----
            # squared denominators for the 5 dots:
            #   [w12, wt0, wt1] need nimg2 * [ntxt2, gt0n2, gt1n2]
            #   [wi0, wi1]      need ntxt2 * [gi0n2, gi1n2]
            den2 = psm.tile([P, 5, G], f32, tag="den2")
            nc.vector.tensor_mul(
                den2[:, 0:3, :],
                stats[:, NIMG : NIMG + 1, :].to_broadcast([P, 3, G]),
                stats[:, NTXT : GT1 + 1, :],
            )
            nc.vector.tensor_mul(
                den2[:, 3:5, :],
                stats[:, NTXT : NTXT + 1, :].to_broadcast([P, 2, G]),
                stats[:, GI0 : GI1 + 1, :],
            )
            den = psm.tile([P, 5, G], f32, tag="den")
            nc.scalar.sqrt(den[:], den2[:])
            nc.vector.tensor_scalar_max(den[:], den[:], EPS)
            rden = psm.tile([P, 5, G], f32, tag="rden")
            nc.vector.reciprocal(rden[:], den[:])
            # cosines: [cos, ct0, ct1, ci0, ci1]
            cosv = psm.tile([P, 5, G], f32, tag="cosv")
            nc.vector.tensor_mul(cosv[:], stats[:, 0:5, :], rden[:])

            out_sb = psm.tile([P, 3 * G], f32, tag="out_sb")
            nc.vector.tensor_copy(out_sb[:, 0:G], cosv[:, 0, :])
            # min distance == max cosine among the two candidates
            cmax = psm.tile([P, 2, G], f32, tag="cmax")
            nc.vector.tensor_max(cmax[:, 0, :], cosv[:, 1, :], cosv[:, 2, :])
            nc.vector.tensor_max(cmax[:, 1, :], cosv[:, 3, :], cosv[:, 4, :])
            # loss rows: relu((cmax - cos) + margin)
            diff = psm.tile([P, 2, G], f32, tag="diff")
            nc.vector.tensor_sub(
                diff[:], cmax[:], cosv[:, 0:1, :].to_broadcast([P, 2, G])
            )
            nc.scalar.activation(
                out_sb[:, G : 3 * G].rearrange("p (k g) -> p k g", k=2),
                diff[:],
                Act.Relu,
                bias=margin[:],
            )
            nc.sync.dma_start(out[:], out_sb[:])

    nc.compile()
    return nc


def _get_nc():
    if "nc" not in _CACHE:
        _CACHE["nc"] = _build()
    return _CACHE["nc"]


def _pack_gidx(cand_shard: np.ndarray) -> np.ndarray:
    """[512, 2] candidate ids -> [128, G*16] int16 in dma_gather's wrapped
    layout: per group g, gather j = k*128 + p fetches cand[g*128+p, k];
    index j lives at [j % 16, g*16 + j // 16], replicated across the 8
    16-partition blocks."""
    out = np.zeros((P, G * 16), dtype=np.int16)
    for g in range(G):
        ids = cand_shard[g * P : (g + 1) * P].T.reshape(2 * P)  # j = k*128+p
        # index j lands at row j%16, col j//16
        blk = np.zeros((16, 16), dtype=np.int16)
        j = np.arange(2 * P)
        blk[j % 16, j // 16] = ids
        out[:, g * 16 : (g + 1) * 16] = np.tile(blk, (8, 1))
    return out


def _make_in_maps(img, txt, cand_img, cand_txt):
    import ml_dtypes

    img_bf = img.astype(ml_dtypes.bfloat16)
    txt_bf = txt.astype(ml_dtypes.bfloat16)
    in_maps = []
    for c in range(NCORES):
        sl = slice(c * RPC, (c + 1) * RPC)
        in_maps.append(
            {
                "img_bf": img_bf,
                "txt_bf": txt_bf,
                "img_anchor": np.ascontiguousarray(img[sl]),
                "txt_anchor": np.ascontiguousarray(txt[sl]),
                "gidx_img": _pack_gidx(cand_img[sl]),
                "gidx_txt": _pack_gidx(cand_txt[sl]),
            }
        )
    return in_maps


def run(img, txt, cand_img, cand_txt, trace=False, trace_kwargs=None):
    """Compile+run on the 8 cores. Returns (loss, cosine, results_obj)."""
    from concourse import bass_utils

    nc = _get_nc()
    in_maps = _make_in_maps(img, txt, cand_img, cand_txt)
    kw = {}
    if trace:
        kw["trace"] = True
        if trace_kwargs:
            kw.update(trace_kwargs)
    res = bass_utils.run_bass_kernel_spmd(
        nc, in_maps, core_ids=list(range(NCORES)), **kw
    )

    cos = np.empty(B, dtype=np.float32)
    i2t_sum = 0.0
    t2i_sum = 0.0
    for c in range(NCORES):
        o = res.results[c]["out"]  # [128, 12]
        cos[c * RPC : (c + 1) * RPC] = o[:, 0:G].T.reshape(RPC)
        i2t_sum += o[:, G : 2 * G].sum(dtype=np.float64)
        t2i_sum += o[:, 2 * G : 3 * G].sum(dtype=np.float64)
    loss = np.float32(i2t_sum / B + t2i_sum / B)
    return loss, cos, res


def kernel(img_embedding, text_embedding, labels, locations, cand_img, cand_txt):
    img = np.ascontiguousarray(np.asarray(img_embedding, dtype=np.float32))
    txt = np.ascontiguousarray(np.asarray(text_embedding, dtype=np.float32))
    ci = np.asarray(cand_img)
    ct = np.asarray(cand_txt)
    loss, cos, _ = run(img, txt, ci, ct)
    return np.array(loss, dtype=np.float32), cos, cos.copy()


# revision 7
# speedup vs baseline: 1.2771x; 1.2771x over previous
"""Trainium2 Bass kernel for contrastive loss with cosine hard-negative mining.

Reference math (B=4096, D=1024):
    loss = mean(relu(pos - i2t_neg + m)) + mean(relu(pos - t2i_neg + m))
    pos      = 1 - cos(img_b, txt_b)
    i2t_neg  = min_k (1 - cos(img_b, txt[cand_txt[b,k]]))   (best-of-2 mining;
    t2i_neg  = min_k (1 - cos(txt_b, img[cand_img[b,k]]))    the mined distance
                                                             IS the neg distance)
    i2t_cosine = t2i_cosine = cos(img_b, txt_b)  (unclamped)

Sharding: data-parallel over batch across 8 cores (512 rows each).
Candidate rows come from bf16 copies of the embedding tables replicated in
each core's HBM, fetched with indirect row-gather DMAs. The bf16 tables are
passed as f32-typed [B, D/2] tensors (same bytes) because the indirect-DMA
path only handles 4-byte elements; compute reads the gathered tiles through
a bf16 bitcast. bf16 halves the gather traffic; the rounding noise averages
out of the loss (rel err ~1e-6) and the exported cosine stays full f32
because the anchor path is f32.

Per 128-row group: 11 fused multiply/square-reduce passes balanced across
ACT and DVE (alternating 6/5 and 5/6 split so both engines carry ~the same
time), then a small epilogue computes cosines, best-candidate mining
(min distance == max candidate cosine) and the relu triplet rows.
Outputs per core: [128, 12] = 4 groups x (cosine, i2t loss row, t2i loss
row). Host concatenates cosine shards and sums the loss partials.
"""

import sys

sys.path.insert(0, "/opt/trn_rl_repo")

import numpy as np

B, D = 4096, 1024
NCORES = 8
RPC = B // NCORES  # rows per core (512)
P = 128
G = RPC // P  # partition groups per core (4)
MARGIN = 0.2
EPS = 1e-8

# stats tile column indices (quantities accumulated per group)
W12, WT0, WT1, WI0, WI1, NIMG, NTXT, GT0, GT1, GI0, GI1 = range(11)

_CACHE = {}


def _build():
    from concourse import bacc, bass, mybir
    import concourse.tile as tile

    dt = mybir.dt
    f32 = dt.float32
    bf16 = dt.bfloat16
    Alu = mybir.AluOpType
    Act = mybir.ActivationFunctionType

    nc = bacc.Bacc("TRN2", target_bir_lowering=False, debug=False)

    # bf16 tables bitcast to f32 pairs for the indirect gather
    img_bfv = nc.dram_tensor("img_bfv", [B, D // 2], f32, kind="ExternalInput")
    txt_bfv = nc.dram_tensor("txt_bfv", [B, D // 2], f32, kind="ExternalInput")
    img_anchor = nc.dram_tensor("img_anchor", [RPC, D], f32, kind="ExternalInput")
    txt_anchor = nc.dram_tensor("txt_anchor", [RPC, D], f32, kind="ExternalInput")
    # packed on host: [p, g*2+k] = cand[g*128+p, k]
    cand_img = nc.dram_tensor("cand_img", [P, G * 2], dt.int32, kind="ExternalInput")
    cand_txt = nc.dram_tensor("cand_txt", [P, G * 2], dt.int32, kind="ExternalInput")
    out = nc.dram_tensor("out", [P, 3 * G], f32, kind="ExternalOutput")

    with tile.TileContext(nc) as tc:
        with (
            tc.tile_pool(name="anchors", bufs=3) as pa,
            tc.tile_pool(name="gathers", bufs=3) as pg,
            tc.tile_pool(name="scratch", bufs=4) as ps,
            tc.tile_pool(name="small", bufs=1) as psm,
        ):
            ci = psm.tile([P, G * 2], dt.int32, tag="ci")
            ct = psm.tile([P, G * 2], dt.int32, tag="ct")
            nc.sync.dma_start(ci[:], cand_img[:])
            nc.sync.dma_start(ct[:], cand_txt[:])

            stats = psm.tile([P, 11, G], f32, tag="stats")

            def act_sq(src, q, g, sdt):
                scr = ps.tile([P, D], sdt, tag=f"scr_act_{sdt.name}")
                nc.scalar.activation(
                    scr[:], src, Act.Square, accum_out=stats[:, q, g : g + 1]
                )

            def dve_dot(x, y, q, g, sdt):
                scr = ps.tile([P, D], sdt, tag=f"scr_dve_{sdt.name}")
                nc.vector.scalar_tensor_tensor(
                    out=scr[:],
                    in0=x,
                    scalar=1.0,
                    in1=y,
                    op0=Alu.mult,
                    op1=Alu.mult,
                    accum_out=stats[:, q, g : g + 1],
                )

            for g in range(G):
                a_img = pa.tile([P, D], f32, tag="a_img")
                a_txt = pa.tile([P, D], f32, tag="a_txt")
                nc.sync.dma_start(a_img[:], img_anchor[g * P : (g + 1) * P, :])
                nc.sync.dma_start(a_txt[:], txt_anchor[g * P : (g + 1) * P, :])

                # gathered candidate rows, bf16 stored as f32 pairs
                gt32 = pg.tile([P, 2, D // 2], f32, tag="gt32")
                gi32 = pg.tile([P, 2, D // 2], f32, tag="gi32")
                for k in range(2):
                    col = g * 2 + k
                    nc.gpsimd.indirect_dma_start(
                        out=gt32[:, k, :],
                        out_offset=None,
                        in_=txt_bfv[:],
                        in_offset=bass.IndirectOffsetOnAxis(
                            ap=ct[:, col : col + 1], axis=0
                        ),
                    )
                    nc.gpsimd.indirect_dma_start(
                        out=gi32[:, k, :],
                        out_offset=None,
                        in_=img_bfv[:],
                        in_offset=bass.IndirectOffsetOnAxis(
                            ap=ci[:, col : col + 1], axis=0
                        ),
                    )
                gt = [gt32[:, k, :].bitcast(bf16) for k in range(2)]
                gi = [gi32[:, k, :].bitcast(bf16) for k in range(2)]

                # anchor-only passes first (can start before gathers land)
                act_sq(a_img[:], NIMG, g, f32)
                act_sq(a_txt[:], NTXT, g, f32)
                dve_dot(a_img[:], a_txt[:], W12, g, f32)
                # candidate passes; alternate the 6th square between the
                # engines so ACT and DVE stay balanced across groups
                act_sq(gt[0], GT0, g, bf16)
                dve_dot(a_img[:], gt[0], WT0, g, bf16)
                act_sq(gt[1], GT1, g, bf16)
                dve_dot(a_img[:], gt[1], WT1, g, bf16)
                act_sq(gi[0], GI0, g, bf16)
                dve_dot(a_txt[:], gi[0], WI0, g, bf16)
                dve_dot(a_txt[:], gi[1], WI1, g, bf16)
                if g % 2 == 0:
                    act_sq(gi[1], GI1, g, bf16)
                else:
                    dve_dot(gi[1], gi[1], GI1, g, bf16)

            margin = psm.tile([P, 1], f32, tag="margin")
            nc.vector.memset(margin[:], MARGIN)

            # ---- epilogue on [P, *, G] slices of stats ----
            # squared denominators for the 5 dots:
            #   [w12, wt0, wt1] need nimg2 * [ntxt2, gt0n2, gt1n2]
            #   [wi0, wi1]      need ntxt2 * [gi0n2, gi1n2]
            den2 = psm.tile([P, 5, G], f32, tag="den2")
            nc.vector.tensor_mul(
                den2[:, 0:3, :],
                stats[:, NIMG : NIMG + 1, :].to_broadcast([P, 3, G]),
                stats[:, NTXT : GT1 + 1, :],
            )
            nc.vector.tensor_mul(
                den2[:, 3:5, :],
                stats[:, NTXT : NTXT + 1, :].to_broadcast([P, 2, G]),
                stats[:, GI0 : GI1 + 1, :],
            )
            den = psm.tile([P, 5, G], f32, tag="den")
            nc.scalar.sqrt(den[:], den2[:])
            nc.vector.tensor_scalar_max(den[:], den[:], EPS)
            rden = psm.tile([P, 5, G], f32, tag="rden")
            nc.vector.reciprocal(rden[:], den[:])
            # cosines: [cos, ct0, ct1, ci0, ci1]
            cosv = psm.tile([P, 5, G], f32, tag="cosv")
            nc.vector.tensor_mul(cosv[:], stats[:, 0:5, :], rden[:])

            out_sb = psm.tile([P, 3 * G], f32, tag="out_sb")
            nc.vector.tensor_copy(out_sb[:, 0:G], cosv[:, 0, :])
            # min distance == max cosine among the two candidates
            cmax = psm.tile([P, 2, G], f32, tag="cmax")
            nc.vector.tensor_max(cmax[:, 0, :], cosv[:, 1, :], cosv[:, 2, :])
            nc.vector.tensor_max(cmax[:, 1, :], cosv[:, 3, :], cosv[:, 4, :])
            # loss rows: relu((cmax - cos) + margin)
            diff = psm.tile([P, 2, G], f32, tag="diff")
            nc.vector.tensor_sub(
                diff[:], cmax[:], cosv[:, 0:1, :].to_broadcast([P, 2, G])
            )
            nc.scalar.activation(
                out_sb[:, G : 3 * G].rearrange("p (k g) -> p k g", k=2),
                diff[:],
                Act.Relu,
                bias=margin[:],
            )
            nc.sync.dma_start(out[:], out_sb[:])

    nc.compile()
    return nc


def _get_nc():
    if "nc" not in _CACHE:
        _CACHE["nc"] = _build()
    return _CACHE["nc"]


def _pack_cand(cand_shard: np.ndarray) -> np.ndarray:
    # [512, 2] -> [128, G*2] with [p, g*2+k] = cand[g*128+p, k]
    return (
        cand_shard.reshape(G, P, 2).transpose(1, 0, 2).reshape(P, G * 2)
        .astype(np.int32)
    )


def _make_in_maps(img, txt, cand_img, cand_txt):
    import ml_dtypes

    img_bfv = np.ascontiguousarray(img.astype(ml_dtypes.bfloat16)).view(np.float32)
    txt_bfv = np.ascontiguousarray(txt.astype(ml_dtypes.bfloat16)).view(np.float32)
    in_maps = []
    for c in range(NCORES):
        sl = slice(c * RPC, (c + 1) * RPC)
        in_maps.append(
            {
                "img_bfv": img_bfv,
                "txt_bfv": txt_bfv,
                "img_anchor": np.ascontiguousarray(img[sl]),
                "txt_anchor": np.ascontiguousarray(txt[sl]),
                "cand_img": _pack_cand(cand_img[sl]),
                "cand_txt": _pack_cand(cand_txt[sl]),
            }
        )
    return in_maps


def run(img, txt, cand_img, cand_txt, trace=False, trace_kwargs=None):
    """Compile+run on the 8 cores. Returns (loss, cosine, results_obj)."""
    from concourse import bass_utils

    nc = _get_nc()
    in_maps = _make_in_maps(img, txt, cand_img, cand_txt)
    kw = {}
    if trace:
        kw["trace"] = True
        if trace_kwargs:
            kw.update(trace_kwargs)
    res = bass_utils.run_bass_kernel_spmd(
        nc, in_maps, core_ids=list(range(NCORES)), **kw
    )

    cos = np.empty(B, dtype=np.float32)
    i2t_sum = 0.0
    t2i_sum = 0.0
    for c in range(NCORES):
        o = res.results[c]["out"]  # [128, 12]
        cos[c * RPC : (c + 1) * RPC] = o[:, 0:G].T.reshape(RPC)
        i2t_sum += o[:, G : 2 * G].sum(dtype=np.float64)
        t2i_sum += o[:, 2 * G : 3 * G].sum(dtype=np.float64)
    loss = np.float32(i2t_sum / B + t2i_sum / B)
    return loss, cos, res


def kernel(img_embedding, text_embedding, labels, locations, cand_img, cand_txt):
    img = np.ascontiguousarray(np.asarray(img_embedding, dtype=np.float32))
    txt = np.ascontiguousarray(np.asarray(text_embedding, dtype=np.float32))
    ci = np.asarray(cand_img)
    ct = np.asarray(cand_txt)
    loss, cos, _ = run(img, txt, ci, ct)
    return np.array(loss, dtype=np.float32), cos, cos.copy()
